# revision 1
# baseline (speedup 1.0000x reference)
"""EntropyAttentionHead Trainium2 kernel.

Per-(b,c) 256-bin histogram over [0,1] -> Shannon entropy -> broadcast to
the spatial map.  Pure data parallel over the 8 NeuronCores: 2048 (b,c)
pairs -> 256 per core.

Histogram strategy (per (b,c), 50176 pixels laid out as [128, 392] in SBUF):
  q  = floor(256*x) in {0..255}   (exact: round-to-int + is_gt fixup)
  ih = q // 16, il = q % 16       (exact in bf16)
  Two 16-plane one-hot tensors (is_equal compares, DVE 4x mode), then the
  256-bin joint histogram is the 16x16 outer-product accumulation
      hist[h,l] = sum_p Hoh[p,h] * Loh[p,l]
  computed by the TensorEngine as accumulating [K,M=16,N=16] matmuls into
  PSUM (fp8 DoubleRow pairs two 128-element chunks per matmul, K=256).
  Entropy tail on ACT/DVE, per-core broadcast of the scalar to the output.
"""

import numpy as np

B, C, H, W = 16, 128, 224, 224
BINS = 256
NPIX = H * W            # 50176
P = 128
NCOLS = NPIX // P       # 392
NCORES = 8
BC_TOTAL = B * C        # 2048
NBC = BC_TOTAL // NCORES  # 256 per core

VARIANT = "fp8drg"      # fp8 DoubleRow + grouped (32-bc) output overlap


def build_nc(nbc=NBC, reps=1, variant=VARIANT):
    import concourse.bacc as bacc
    import concourse.bass as bass
    import concourse.tile as tile
    from concourse import mybir

    f32 = mybir.dt.float32
    bf16 = mybir.dt.bfloat16
    fp8 = mybir.dt.float8e4
    i32 = mybir.dt.int32
    OP = mybir.AluOpType
    AF = mybir.ActivationFunctionType
    MM = mybir.MatmulPerfMode

    mh, nl = 16, 16
    if variant.startswith("fp8dr"):
        ncols = 416           # pad 392 -> 416 = 2*208 for DoubleRow pairing
        half = ncols // 2
        oh_dt = fp8
        if variant == "fp8dr832":
            mh, nl = 8, 32
        grp = 32 if variant == "fp8drg" else 0
    else:
        grp = 0
        ncols = NCOLS
        half = 0
        oh_dt = bf16
        if variant == "bf16_8x32":
            mh, nl = 8, 32
        elif variant == "bf16_32x8":
            mh, nl = 32, 8
        elif variant in ("fp8", "fp8_constw"):
            oh_dt = fp8
    nplanes = mh + nl
    inv_nl = 1.0 / float(nl)

    nc = bacc.Bacc("TRN2", target_bir_lowering=False, debug=False)
    x_d = nc.dram_tensor("x", [nbc, P, NCOLS], f32, kind="ExternalInput").ap()
    o_d = nc.dram_tensor("o", [nbc, P, NCOLS], f32, kind="ExternalOutput").ap()

    inv_n = 1.0 / float(NPIX)

    with tile.TileContext(nc) as tc:
        with (
            tc.tile_pool(name="xin", bufs=3) as xin_p,
            tc.tile_pool(name="prep", bufs=3) as prep_p,
            tc.tile_pool(name="oh", bufs=3 if variant == "fp8dr2" else 2) as oh_p,
            tc.tile_pool(name="ps", bufs=6 if variant == "fp8dr2" else 4,
                         space="PSUM") as ps_p,
            tc.tile_pool(name="tail", bufs=4) as tail_p,
            tc.tile_pool(name="fin", bufs=1) as fin_p,
            tc.tile_pool(name="dram", bufs=2, space="DRAM") as dram_p,
            tc.tile_pool(name="outp", bufs=3) as out_p,
            tc.tile_pool(name="pse", bufs=2, space="PSUM") as pse_p,
        ):
            ebuf = fin_p.tile([mh, nbc], f32)
            eps16 = fin_p.tile([mh, 1], f32)
            nc.vector.memset(eps16, 1e-10)
            ones16 = fin_p.tile([mh, 1], f32)
            nc.vector.memset(ones16, 1.0)
            dz = fin_p.tile([P, NCOLS], f32)
            nc.vector.memset(dz, 0.0)
            cw = fin_p.tile([P, 16], fp8)
            nc.vector.memset(cw, 1.0)

            def body():
                for ibc in range(nbc):
                    xt = xin_p.tile([P, ncols], f32, tag="xt")
                    nc.sync.dma_start(out=xt[:, 0:NCOLS], in_=x_d[ibc])
                    if ncols > NCOLS:
                        # pad -> ih=32 (out of range) -> zero H one-hot
                        nc.vector.memset(xt[:, NCOLS:ncols], 2.0)

                    # q = floor(256 x): r = round_i32(256x); q = r - (r > 256x)
                    t = prep_p.tile([P, ncols], f32, tag="t")
                    nc.vector.tensor_scalar(
                        out=t, in0=xt, scalar1=256.0, scalar2=None, op0=OP.mult)
                    ri = prep_p.tile([P, ncols], i32, tag="ri")
                    nc.vector.tensor_copy(out=ri, in_=t)
                    r = prep_p.tile([P, ncols], f32, tag="r")
                    nc.vector.tensor_copy(out=r, in_=ri)
                    adj = prep_p.tile([P, ncols], f32, tag="adj")
                    nc.vector.tensor_tensor(out=adj, in0=r, in1=t, op=OP.is_gt)
                    q = prep_p.tile([P, ncols], bf16, tag="q")
                    nc.vector.tensor_tensor(out=q, in0=r, in1=adj, op=OP.subtract)
                    # ih = floor(q/nl) same trick (bf16 exact); il = q - nl*ih
                    u = prep_p.tile([P, ncols], bf16, tag="u")
                    nc.vector.tensor_scalar(
                        out=u, in0=q, scalar1=inv_nl, scalar2=None, op0=OP.mult)
                    ui = prep_p.tile([P, ncols], i32, tag="ui")
                    nc.vector.tensor_copy(out=ui, in_=u)
                    r2 = prep_p.tile([P, ncols], bf16, tag="r2")
                    nc.vector.tensor_copy(out=r2, in_=ui)
                    adj2 = prep_p.tile([P, ncols], bf16, tag="adj2")
                    nc.vector.tensor_tensor(out=adj2, in0=r2, in1=u, op=OP.is_gt)
                    ih = prep_p.tile([P, ncols], bf16, tag="ih")
                    nc.vector.tensor_tensor(out=ih, in0=r2, in1=adj2, op=OP.subtract)
                    il = prep_p.tile([P, ncols], bf16, tag="il")
                    nc.vector.scalar_tensor_tensor(
                        out=il, in0=ih, scalar=-float(nl), in1=q,
                        op0=OP.mult, op1=OP.add)

                    # one-hot planes [128, mh+nl, ncols]; 0..mh-1 = ih planes
                    oh = oh_p.tile([P, nplanes, ncols], oh_dt, tag="oh")
                    for j in range(mh):
                        nc.vector.tensor_scalar(
                            out=oh[:, j, :], in0=ih, scalar1=float(j),
                            scalar2=None, op0=OP.is_equal)
                    for j in range(nl):
                        nc.vector.tensor_scalar(
                            out=oh[:, mh + j, :], in0=il, scalar1=float(j),
                            scalar2=None, op0=OP.is_equal)

                    # joint histogram: accumulating matmuls
                    ps = ps_p.tile([mh, nl], f32, tag="ps")
                    if variant.startswith("fp8dr"):
                        base = oh[:, :, :]
                        p0 = list(base.ap[0])
                        for n in range(half):
                            lhsT = bass.AP(
                                tensor=base.tensor, offset=base.offset + n,
                                ap=[p0, [half, 2], [ncols, mh]])
                            rhs = bass.AP(
                                tensor=base.tensor,
                                offset=base.offset + mh * ncols + n,
                                ap=[p0, [half, 2], [ncols, nl]])
                            nc.tensor.matmul(
                                out=ps, lhsT=lhsT, rhs=rhs,
                                start=(n == 0), stop=(n == half - 1),
                                perf_mode=MM.DoubleRow)
                    elif variant == "fp8_constw":
                        # TIMING PROBE ONLY: contiguous constant weights (FWL)
                        for n in range(ncols):
                            nc.tensor.matmul(
                                out=ps, lhsT=cw,
                                rhs=oh[:, mh:nplanes, n:n + 1],
                                start=(n == 0), stop=(n == ncols - 1))
                    else:
                        for n in range(ncols):
                            nc.tensor.matmul(
                                out=ps,
                                lhsT=oh[:, 0:mh, n:n + 1],
                                rhs=oh[:, mh:nplanes, n:n + 1],
                                start=(n == 0), stop=(n == ncols - 1))

                    # entropy tail: sum p*ln(p + 1e-10), p = c/NPIX
                    u2 = tail_p.tile([mh, nl], f32, tag="u2")
                    nc.scalar.activation(
                        out=u2, in_=ps, func=AF.Ln, bias=eps16, scale=inv_n)
                    term = tail_p.tile([mh, nl], f32, tag="term")
                    nc.vector.scalar_tensor_tensor(
                        out=term, in0=ps, scalar=inv_n, in1=u2,
                        op0=OP.mult, op1=OP.mult)
                    nc.vector.tensor_reduce(
                        out=ebuf[:, ibc:ibc + 1], in_=term,
                        axis=mybir.AxisListType.XYZW, op=OP.add)

                    if grp and (ibc + 1) % grp == 0:
                        g0 = ibc + 1 - grp
                        pseg = pse_p.tile([1, grp], f32, tag="pseg")
                        nc.tensor.matmul(out=pseg, lhsT=ones16,
                                         rhs=ebuf[:, g0:ibc + 1],
                                         start=True, stop=True)
                        esbg = tail_p.tile([1, grp], f32, tag="esbg")
                        nc.scalar.activation(out=esbg, in_=pseg,
                                             func=AF.Copy, scale=-1.0)
                        edg = dram_p.tile([1, grp], f32, tag="edg")
                        nc.sync.dma_start(out=edg, in_=esbg)
                        e128g = tail_p.tile([P, grp], f32, tag="e128g")
                        bc_ap = bass.AP(
                            tensor=edg.tensor, offset=edg.offset,
                            ap=[[0, P], list(edg.ap[-1])])
                        nc.sync.dma_start(out=e128g, in_=bc_ap)
                        for k in range(grp):
                            ot = out_p.tile([P, NCOLS], f32, tag="ot")
                            nc.scalar.activation(
                                out=ot, in_=dz, func=AF.Identity,
                                bias=e128g[:, k:k + 1], scale=0.0)
                            nc.sync.dma_start(out=o_d[g0 + k], in_=ot)

                if grp:
                    return
                # reduce over mh partitions with a ones-matmul, negate
                pse = pse_p.tile([1, nbc], f32, tag="pse")
                nc.tensor.matmul(out=pse, lhsT=ones16, rhs=ebuf,
                                 start=True, stop=True)
                esb = fin_p.tile([1, nbc], f32, tag="esb")
                nc.scalar.activation(out=esb, in_=pse, func=AF.Copy, scale=-1.0)

                # broadcast to 128 partitions via DRAM roundtrip
                edram = dram_p.tile([1, nbc], f32, tag="edram")
                nc.sync.dma_start(out=edram, in_=esb)
                e128 = fin_p.tile([P, nbc], f32, tag="e128")
                bcast = bass.AP(
                    tensor=edram.tensor, offset=edram.offset,
                    ap=[[0, P], list(edram.ap[-1])])
                nc.sync.dma_start(out=e128, in_=bcast)

                for ibc in range(nbc):
                    ot = out_p.tile([P, NCOLS], f32, tag="ot")
                    nc.scalar.activation(
                        out=ot, in_=dz, func=AF.Identity,
                        bias=e128[:, ibc:ibc + 1], scale=0.0)
                    nc.sync.dma_start(out=o_d[ibc], in_=ot)

            if reps == 1:
                body()
            else:
                with tc.For_i(0, reps):
                    body()

    nc.finalize()
    return nc


_NC_CACHE = {}


def _get_nc(key):
    if key not in _NC_CACHE:
        _NC_CACHE[key] = build_nc(*key)
    return _NC_CACHE[key]


def run_sharded(x_r, nbc=NBC, reps=1, variant=VARIANT):
    """x_r: [ncores*nbc, P, NCOLS] float32 -> same-shape output."""
    from concourse.bass_utils import run_bass_kernel_spmd

    nc = _get_nc((nbc, reps, variant))
    ncores = x_r.shape[0] // nbc
    in_maps = [
        {"x": np.ascontiguousarray(x_r[i * nbc:(i + 1) * nbc])}
        for i in range(ncores)
    ]
    res = run_bass_kernel_spmd(nc, in_maps, core_ids=list(range(ncores)))
    out = np.concatenate([r["o"] for r in res.results], axis=0)
    return out


def kernel(x, bins):
    assert int(bins) == BINS
    x = np.asarray(x, dtype=np.float32)
    assert x.shape == (B, C, H, W), x.shape
    x_r = x.reshape(BC_TOTAL, P, NCOLS)
    out = run_sharded(x_r, NBC)
    return out.reshape(B, C, H, W).astype(np.float32)

